# revision 1
# baseline (speedup 1.0000x reference)
"""Trainium2 Bass kernel for nn_ConvBlock_23021024707487 (v2: fp8 DoubleRow).

Binarized double conv-block + residual + maxpool, data-parallel over batch
across 8 NeuronCores (2 images per core).

v2 strategy:
- Phase A (conv1x1 on real x): unchanged bf16 4-piece int8 decomposition of
  round(x*2^28) -- proven bit-exact.
- Phases B/D (binary conv3x3) in fp8e5 DoubleRow: 4 matmuls per output tile.
  Partitions 64-127 hold a +1-column-shifted copy (sigma trick); the DoubleRow
  plane dim (stride = row pitch) pairs vertically adjacent taps. Invalid taps
  get zero weights.
- Phase C (binary conv1x1, K=256) in fp8e5 DoubleRow: 1 matmul per tile
  (planes = channel halves).
- Residual: an extra DoubleRow matmul adds BIG*h into phase D's PSUM, so
  binarize(h+r) becomes a single threshold test after maxpool (max commutes
  with the monotone threshold).
- Binarize ops split across engines: ACT produces +-1 (Sign), DVE produces
  0/1 (is_ge vs half-integer thresholds). Conventions are folded exactly on
  host: x2 weights + threshold shifts for 0/1 inputs, padding stored as 0.5
  so the +-1 <-> 0/1 affine map holds at borders too. All post-A PSUM values
  are integers, so half-integer ("midpoint") thresholds make every
  comparison exact in fp32.
"""

import sys

for _p in ("/opt/trn_rl_repo", "/root/.axon_site/_ro/trn_rl_repo"):
    if _p not in sys.path:
        sys.path.insert(0, _p)

import numpy as np
import ml_dtypes

import concourse.bacc as bacc
import concourse.mybir as mybir
from concourse import tile
from concourse.ap import AP
from concourse.bass_utils import run_bass_kernel_spmd

BF16 = mybir.dt.bfloat16
F32 = mybir.dt.float32
F32R = mybir.dt.float32r
FP16 = mybir.dt.float16
FP8 = mybir.dt.float8e5
NPBF16 = ml_dtypes.bfloat16
NPFP8 = ml_dtypes.float8_e5m2
DR = mybir.MatmulPerfMode.DoubleRow
SIGN = mybir.ActivationFunctionType.Sign
GE = mybir.AluOpType.is_ge

N_CORES = 8
B, CIN, DOWN, UP, H, W = 16, 256, 64, 256, 56, 56
HW = H * W               # 3136
PWP = 64                 # padded row pitch (bytes, fp8)
PH = 59                  # 58 real padded rows + 1 slack row (OOB-safe reads)
PBUF = PH * PWP          # 3776
IMGS = B // N_CORES      # 2
ROWS = 8
NT = H // ROWS           # 7
NTILE = ROWS * W         # 448
EPS = 1e-4
QBITS = 28
NPIECES = 4
BIG = 4096.0             # resid fold scale for 0/1 h; 2048 for +-1 h

_compiled = None


def _sign(w):
    return np.where(w >= 0, 1.0, -1.0)


def _ov(t_ap, off, dims):
    """Hand-built (possibly overlapping) AP on an SBUF tile."""
    return AP(t_ap.tensor, t_ap.offset + off,
              [list(t_ap.ap[0])] + [list(d) for d in dims])


def _build_nc():
    nc = bacc.Bacc("TRN2", target_bir_lowering=False, debug=False,
                   num_devices=N_CORES)

    xp = nc.declare_dram_parameter("xp", [IMGS, 3, 2, 128, HW],
                                   mybir.dt.float16, isOutput=False)
    wa = nc.declare_dram_parameter("wa", [128, 128], mybir.dt.float16,
                                   isOutput=False)
    # w8 layout (fp8e5, free dim):
    #  0:2048    wB  [mh(2), mm(4), plane(2), 128]
    #  2048:4096 wD  [mh(2), mm(4), plane(2), 128]
    #  4096:4608 wR  [mh(2), plane(2), 128]   resid identity blocks
    #  4608:4736 wC  [plane(2), 64]
    w8 = nc.declare_dram_parameter("w8", [128, 4736], FP8, isOutput=False)
    # bn/threshold table f32, columns:
    #  0 inv11/2^28 | 1 beta11 | 2,3 inv31,beta31 mh0 | 4,5 mh1
    #  6 tC* | 7,8 tD* mh0,mh1 (negated for ACT bias use where needed)
    bnp = nc.declare_dram_parameter("bn", [128, 16], F32, isOutput=False)
    y = nc.declare_dram_parameter("y", [IMGS, UP, H // 2, W // 2], F32,
                                  isOutput=True)

    with tile.TileContext(nc) as tc:
        with (
            tc.tile_pool(name="const", bufs=1) as cpool,
            tc.tile_pool(name="act", bufs=1) as apool,
            tc.tile_pool(name="work", bufs=4) as wpool,
            tc.tile_pool(name="psA", bufs=2, space="PSUM") as psA,
            tc.tile_pool(name="psB", bufs=2, space="PSUM") as psB,
            tc.tile_pool(name="psD", bufs=3, space="PSUM") as psD,
        ):
            bn_sb = cpool.tile([128, 16], F32, tag="bn")
            nc.sync.dma_start(out=bn_sb[:], in_=bnp[:])
            wa_sb = cpool.tile([128, 128], mybir.dt.float16, tag="wa")
            nc.sync.dma_start(out=wa_sb[:], in_=wa[:])
            w8_sb = cpool.tile([128, 4736], FP8, tag="w8")
            nc.sync.dma_start(out=w8_sb[:], in_=w8[:])



            w1a = [wa_sb[:, kh * 64:kh * 64 + 64] for kh in range(2)]

            def w3(base, mh, j):          # [128, 2, 128] conv3x3 weight block
                o = base + (mh * 4 + j) * 256
                return w8_sb[:, o:o + 256].rearrange("p (two m) -> p two m",
                                                     two=2)

            def wR(mh):                   # [128, 2, 128] resid identity block
                o = 4096 + mh * 256
                return w8_sb[:, o:o + 256].rearrange("p (two m) -> p two m",
                                                     two=2)

            wC = w8_sb[:, 4608:4736].rearrange("p (two m) -> p two m", two=2)

            def bncol(c, p=128):
                return bn_sb[0:p, c:c + 1]

            # ---- persistent activation buffers ----
            xsb = [[[apool.tile([128, HW], mybir.dt.float16,
                                tag=f"xsb{i}{k}{kh}", name=f"xsb{i}{k}{kh}")
                     for kh in range(2)] for k in range(3)]
                   for i in range(IMGS)]
            for i in range(IMGS):
                for k in range(3):
                    for kh in range(2):
                        for c in range(2):
                            sl = slice(c * HW // 2, (c + 1) * HW // 2)
                            nc.sync.dma_start(out=xsb[i][k][kh][:, sl],
                                              in_=xp[i, k, kh][:, sl])
            x1p = [apool.tile([128, PBUF], FP8, tag=f"x1p{i}", name=f"x1p{i}")
                   for i in range(IMGS)]
            x2p = [apool.tile([128, PBUF], FP8, tag=f"x2p{i}", name=f"x2p{i}")
                   for i in range(IMGS)]
            # hbuf plane dim: 0 = ch 0-127 (+-1, ACT), 1 = ch 128-255 (0/1 DVE)
            hbuf = [apool.tile([128, 2, HW], FP8, tag=f"h{i}", name=f"h{i}")
                    for i in range(IMGS)]
            obuf = [[apool.tile([128, HW // 4], F32, tag=f"o{i}{m}",
                                name=f"o{i}{m}") for m in range(2)]
                    for i in range(IMGS)]

            # border zeroing: rows 0,57,58; col 0; cols 57-58 (col 58 feeds
            # the sigma copy into col 57 of partitions 64-127). x2p borders
            # are 0.5 (0/1 convention), x1p borders 0.0 (+-1 convention).
            for t, pad in ((x1p[0], 0.0), (x1p[1], 0.0),
                           (x2p[0], 0.0), (x2p[1], 0.0)):
                t3 = t[:].rearrange("p (h w) -> p h w", w=PWP)
                nc.gpsimd.memset(t[:, 0:PWP], pad)                # row 0
                nc.gpsimd.memset(t[:, 57 * PWP:PBUF], pad)        # rows 57-58
                nc.gpsimd.memset(t3[:, 1:57, 0:1], pad)           # col 0
                nc.gpsimd.memset(t3[:, 1:57, 57:59], pad)         # cols 57-58

            SHALF = 30 * PWP

            def sigma_copy(dst, half):
                # partitions 64-127 <- partitions 0-63 shifted left 1 byte
                if half == 0:
                    nc.sync.dma_start(out=dst[64:128, 0:SHALF],
                                      in_=dst[0:64, 1:SHALF + 1])
                else:
                    nc.sync.dma_start(out=dst[64:128, SHALF:PBUF - 1],
                                      in_=dst[0:64, SHALF + 1:PBUF])

            def conv3x3(src, wbase, mh, t, pspool, pstag, extra=None):
                """4 DoubleRow matmuls; optional 5th accumulating matmul."""
                r0 = t * ROWS
                ps = pspool.tile([128, NTILE], F32, tag=pstag)
                sap = src[:]
                offs = ((r0, 0), (r0 + 2, 0), (r0, 2), (r0 + 2, 2))
                for j, (dr, dc) in enumerate(offs):
                    rhs = _ov(sap, dr * PWP + dc,
                              [[PWP, 2], [PWP, ROWS], [1, W]])
                    nc.tensor.matmul(ps[:], w3(wbase, mh, j), rhs,
                                     start=(j == 0),
                                     stop=(j == 3 and extra is None),
                                     perf_mode=DR)
                if extra is not None:
                    nc.tensor.matmul(ps[:], wR(mh),
                                     extra[:, 0:2, t * NTILE:(t + 1) * NTILE],
                                     start=False, stop=True, perf_mode=DR)
                return ps

            def phase_A(img, t):
                c0 = t * NTILE
                ps = psA.tile([64, NTILE], F32, tag="pa")
                n = 0
                for k in range(3):
                    for kh in range(2):
                        nc.tensor.matmul(ps[:], w1a[kh],
                                         xsb[img][k][kh][:, c0:c0 + NTILE],
                                         start=(n == 0), stop=(n == 5))
                        n += 1
                d3 = x1p[img][:].rearrange("p (h w) -> p h w", w=PWP)
                r0 = t * ROWS
                nc.scalar.activation(d3[0:64, r0 + 1:r0 + 9, 1:57], ps[:],
                                     SIGN, bias=bncol(1, 64),
                                     scale=bncol(0, 64))

            def phase_B(img, t, mh):
                ps = conv3x3(x1p[img], 0, mh, t, psB, "pb")
                dst = hbuf[img][:, mh, t * NTILE:(t + 1) * NTILE]
                if mh == 0:
                    nc.scalar.activation(dst, ps[:], SIGN,
                                         bias=bncol(3), scale=bncol(2))
                else:
                    nc.vector.tensor_scalar(out=dst, in0=ps[:],
                                            scalar1=bncol(4), scalar2=None,
                                            op0=GE)

            def phase_C(img, t):
                c0 = t * NTILE
                ps = psA.tile([64, NTILE], F32, tag="pa")
                nc.tensor.matmul(ps[:], wC, hbuf[img][:, 0:2, c0:c0 + NTILE],
                                 start=True, stop=True, perf_mode=DR)
                d3 = x2p[img][:].rearrange("p (h w) -> p h w", w=PWP)
                r0 = t * ROWS
                nc.scalar.activation(d3[0:64, r0 + 1:r0 + 9, 1:57], ps[:],
                                     SIGN, bias=bncol(10, 64),
                                     scale=bncol(6, 64))

            def phase_D(img, t, mh):
                ps = conv3x3(x2p[img], 2048, mh, t, psD, "pd",
                             extra=hbuf[img])
                # binarize (resid already folded into PSUM), then maxpool in
                # SBUF. Binarize alternates ACT (+-1) / DVE (0/1) by tile
                # parity to balance engines; final Sign bias handles both.
                s = wpool.tile([128, NTILE], BF16, tag="s")
                if t % 2 == 0:
                    nc.scalar.activation(s[:], ps[:], SIGN,
                                         bias=bncol(11 + mh), scale=1.0)
                else:
                    nc.vector.tensor_scalar(out=s[:], in0=ps[:],
                                            scalar1=bncol(7 + mh),
                                            scalar2=None, op0=GE)
                s4 = s[:].rearrange("p (h two w) -> p h two w", two=2, w=W)
                v = wpool.tile([128, ROWS // 2, W], BF16, tag="v")
                nc.vector.tensor_max(out=v[:], in0=s4[:, :, 0, :],
                                     in1=s4[:, :, 1, :])
                v4 = v[:].rearrange("p h (w two) -> p h w two", two=2)
                hm = wpool.tile([128, ROWS // 2, W // 2], BF16, tag="hm")
                nc.vector.tensor_max(out=hm[:], in0=v4[:, :, :, 0],
                                     in1=v4[:, :, :, 1])
                c = t * (ROWS // 2) * (W // 2)
                dst = obuf[img][mh][:, c:c + 112].rearrange(
                    "p (h w) -> p h w", w=W // 2)
                if t % 2 == 0:   # hm is +-1 already: plain convert
                    nc.vector.tensor_copy(out=dst, in_=hm[:])
                else:            # hm is 0/1: map to 2v-1
                    nc.vector.tensor_scalar(out=dst, in0=hm[:], scalar1=2.0,
                                            scalar2=-1.0, op0=mybir.AluOpType.mult,
                                            op1=mybir.AluOpType.add)

            def store_out(img, mh, half):
                hh = HW // 8
                sl = slice(half * hh, (half + 1) * hh)
                nc.sync.dma_start(
                    out=y[img, mh * 128:(mh + 1) * 128].rearrange(
                        "p h w -> p (h w)")[:, sl],
                    in_=obuf[img][mh][:, sl])

            # Schedule (same interleave shape as baseline)
            for t in range(NT):
                phase_A(0, t)
                if t == 4:
                    sigma_copy(x1p[0], 0)
            sigma_copy(x1p[0], 1)
            for t in range(NT):
                phase_B(0, t, 0)
                phase_A(1, t)
                if t == 4:
                    sigma_copy(x1p[1], 0)
            sigma_copy(x1p[1], 1)
            for t in range(NT):
                phase_B(0, t, 1)
                phase_B(1, t, 0)
            for t in range(NT):
                phase_C(0, t)
                phase_B(1, t, 1)
                if t == 4:
                    sigma_copy(x2p[0], 0)
            sigma_copy(x2p[0], 1)
            for t in range(NT):
                phase_D(0, t, 0)
                phase_C(1, t)
                if t == 4:
                    sigma_copy(x2p[1], 0)
            sigma_copy(x2p[1], 1)
            for t in range(NT):
                phase_D(0, t, 1)
                phase_D(1, t, 0)
                phase_D(1, t, 1)
                if t == 4:
                    store_out(0, 0, 0)
                    store_out(0, 0, 1)
                    store_out(0, 1, 0)
                    store_out(1, 0, 0)
                    store_out(1, 1, 0)
            store_out(0, 1, 1)
            store_out(1, 0, 1)
            store_out(1, 1, 1)

    nc.compile()
    return nc


def _midpoint(t):
    """Half-integer strictly-equivalent threshold for integer-valued PSUM:
    v >= t  <=>  v >= ceil(t) (non-integer t) / v >= t (integer t)."""
    t = np.asarray(t, np.float64)
    up = np.ceil(t)
    up = np.where(up == t, t, up)      # integer t: keep (inclusive >=)
    return (up - 0.5).astype(np.float32)


def _host_prep(inputs):
    f64 = np.float64

    def inv_beta(g, b, m, v):
        inv = g.astype(f64) / np.sqrt(v.astype(f64) + EPS)
        return inv, b.astype(f64) - m.astype(f64) * inv

    inv11, beta11 = inv_beta(inputs["g11"], inputs["b11"], inputs["m11"], inputs["v11"])
    inv31, beta31 = inv_beta(inputs["g31"], inputs["b31"], inputs["m31"], inputs["v31"])
    inv12, beta12 = inv_beta(inputs["g12"], inputs["b12"], inputs["m12"], inputs["v12"])
    inv32, beta32 = inv_beta(inputs["g32"], inputs["b32"], inputs["m32"], inputs["v32"])

    W31 = _sign(inputs["w31"])           # [256, 64, 3, 3]
    W12 = _sign(inputs["w12"][:, :, 0, 0])   # [64, 256]
    W32 = _sign(inputs["w32"])

    # ---- w8 blob ----
    w8 = np.zeros((128, 4736), f64)

    def pack3(base, Wf, wscale):
        # Wf [256, 64, 3, 3]; lhsT[k, plane, m]: partitions 0-63 tap kx,
        # 64-127 tap kx+1; mm offsets (ky base, kx base):
        # mm0 (0,0): plane p -> ky=p, kx 0/1
        # mm1 (2,0): plane0 ky=2 kx 0/1, plane1 zero
        # mm2 (0,2): plane p -> ky=p, kx=2 (parts 0-63 only)
        # mm3 (2,2): plane0 ky=2 kx=2 (parts 0-63 only)
        for mh in range(2):
            ms = slice(mh * 128, (mh + 1) * 128)
            blk = np.zeros((4, 128, 2, 128), f64)
            blk[0, 0:64, 0] = Wf[ms, :, 0, 0].T * wscale
            blk[0, 64:128, 0] = Wf[ms, :, 0, 1].T * wscale
            blk[0, 0:64, 1] = Wf[ms, :, 1, 0].T * wscale
            blk[0, 64:128, 1] = Wf[ms, :, 1, 1].T * wscale
            blk[1, 0:64, 0] = Wf[ms, :, 2, 0].T * wscale
            blk[1, 64:128, 0] = Wf[ms, :, 2, 1].T * wscale
            blk[2, 0:64, 0] = Wf[ms, :, 0, 2].T * wscale
            blk[2, 0:64, 1] = Wf[ms, :, 1, 2].T * wscale
            blk[3, 0:64, 0] = Wf[ms, :, 2, 2].T * wscale
            for j in range(4):
                o = base + (mh * 4 + j) * 256
                w8[:, o:o + 256] = blk[j].reshape(128, 256)

    pack3(0, W31, 1.0)      # B: x1p is +-1
    pack3(2048, W32, 1.0)   # D: x2p is +-1 (ACT-produced)

    # resid identity: plane mh = scale*I (h mh0 +-1 -> 2048; mh1 0/1 -> 4096)
    for mh, s in ((0, BIG / 2), (1, BIG)):
        o = 4096 + mh * 256
        blk = np.zeros((128, 2, 128), f64)
        blk[:, mh, :] = np.eye(128) * s
        w8[:, o:o + 256] = blk.reshape(128, 256)

    # C: plane0 ch0-127 (+-1, x1), plane1 ch128-255 (0/1, x2)
    wc = np.zeros((128, 2, 64), f64)
    wc[:, 0] = W12[:, 0:128].T
    wc[:, 1] = W12[:, 128:256].T * 2.0
    w8[:, 4608:4736] = wc.reshape(128, 128)

    # ---- phase A: fp16 weights (+-1), applied to 3 fp16 x-terms ----
    W1 = _sign(inputs["w11"][:, :, 0, 0]).T          # [256, 64]
    wa = np.zeros((128, 128), np.float16)
    for kh in range(2):
        wa[:, kh * 64:kh * 64 + 64] = W1[kh * 128:(kh + 1) * 128]

    # ---- bn/threshold table ----
    bn = np.zeros((128, 16), np.float32)
    bn[0:64, 0] = inv11.astype(np.float32)
    bn[0:64, 1] = beta11.astype(np.float32)
    bn[:, 2] = inv31[0:128].astype(np.float32)
    bn[:, 3] = beta31[0:128].astype(np.float32)
    bn[:, 4] = _midpoint(-beta31[128:256] / inv31[128:256])
    # C (ACT Sign): raw = sum_lo W h+- + sum_hi 2W h01; TRUE = raw - S_hi
    # presign = inv12*TRUE + beta12 = raw*inv12 + (beta12 - inv12*S_hi)
    S_hi = W12[:, 128:256].sum(axis=1)
    bn[0:64, 6] = inv12.astype(np.float32)
    bn[0:64, 10] = (beta12 - inv12 * S_hi).astype(np.float32)
    # D: raw = conv(W, x2p +-1, pad 0) + resid; t = -beta32/inv32
    tDm = _midpoint(-beta32 / inv32)
    # D is_ge thresholds on q = raw' + resid (mh0: h is +-1 -> scale 2048,
    # threshold shifts by -2048; mh1: h is 0/1 -> scale 4096, no shift)
    bn[:, 7] = tDm[0:128] - BIG / 2    # DVE is_ge thresholds
    bn[:, 8] = tDm[128:256]
    bn[:, 9] = -0.5                    # final map bias (works for 0/1 & +-1)
    bn[:, 11] = -(tDm[0:128] - BIG / 2)   # ACT Sign biases for D binarize
    bn[:, 12] = -tDm[128:256]

    # ---- x = h1 + h2 + h3, each fp16 (33 significand bits total) ----
    x = inputs["x"].astype(f64)
    h1 = x.astype(np.float16)
    h2 = (x - h1.astype(f64)).astype(np.float16)
    h3 = (x - h1.astype(f64) - h2.astype(f64)).astype(np.float16)
    resid = np.abs(x - h1.astype(f64) - h2.astype(f64) - h3.astype(f64))
    assert resid.max() < 4e-8, resid.max()
    terms = (h1, h2, h3)

    w8_8 = w8.astype(NPFP8)
    assert np.array_equal(w8_8.astype(f64), w8), "w8 not fp8e5-exact"

    in_maps = []
    for c in range(N_CORES):
        xs = np.zeros((IMGS, 3, 2, 128, HW), np.float16)
        for i in range(IMGS):
            img = c * IMGS + i
            for k in range(3):
                tc2 = terms[k][img].reshape(CIN, HW)
                xs[i, k, 0] = tc2[0:128]
                xs[i, k, 1] = tc2[128:256]
        in_maps.append({"xp": xs, "wa": wa, "w8": w8_8, "bn": bn})
    return in_maps


def kernel(**inputs):
    global _compiled
    if _compiled is None:
        _compiled = _build_nc()
    in_maps = _host_prep(inputs)
    res = run_bass_kernel_spmd(_compiled, in_maps, list(range(N_CORES))).results
    out = np.concatenate([res[c]["y"] for c in range(N_CORES)], axis=0)
    return out.astype(np.float32)



# revision 77
# speedup vs baseline: 1.3825x; 1.3825x over previous
"""Trainium2 Bass kernel for nn_ConvBlock_23021024707487 (v3).

Binarized double conv-block + residual + maxpool, data-parallel over batch
across 8 NeuronCores (2 images per core).

v3 strategy (vs v2):
- Phase A (conv1x1 on real x): f32r matmuls directly on raw f32 x
  (cost model: 1 cycle/row at N>=448; interp numerics = plain f32).
  2 matmuls per tile instead of 6, and input DMA drops from 9.6MB
  (3x fp16 split) to 6.4MB (raw f32).
- conv3x3 in 3 DoubleRow matmuls instead of 4: mm0 covers taps
  (ky,kx) in {0,1}x{0,1} (planes=ky via row stride, partitions=kx via
  sigma copy), mm1 covers (0,2),(1,2) on partitions 0-63, mm2 uses
  plane stride of 2 BYTES (kx+0 / kx+2) to cover (2,0),(2,1),(2,2).
- Phase D: maxpool runs on PSUM *before* thresholding (max commutes with
  the monotone threshold): DVE row-pair max from PSUM, Pool-engine
  col-pair max, one ACT Sign per (img,mh,half) -> fp8 obuf.
- Residual folded into phase-D PSUM via an identity DoubleRow matmul
  (BIG*h), so binarize(h+r) + maxpool collapses into threshold(maxpool(q)).
- Engine balance: ACT does A/Bmh0 binarize + D finals; DVE does Bmh1
  (is_ge 0/1) + D row-pool; Pool does C binarize (is_ge 0/1) + D col-pool
  + border memsets.
- Conventions: x1p is +-1 (pad 0), x2p is 0/1 (pad 0.5, D weights 2W with
  threshold shift by rowsum(W)), hbuf plane0 +-1 / plane1 0/1 (folded in
  wC and the resid scales). All post-A PSUM values are integers, so
  half-integer thresholds make every comparison exact in fp32.
- Output y is fp8 (+-1 exact); host converts to f32.
"""

import sys

for _p in ("/opt/trn_rl_repo", "/root/.axon_site/_ro/trn_rl_repo"):
    if _p not in sys.path:
        sys.path.insert(0, _p)

import numpy as np
import ml_dtypes

import concourse.bacc as bacc
import concourse.mybir as mybir
from concourse import tile
from concourse.ap import AP
from concourse.bass_utils import run_bass_kernel_spmd

F32 = mybir.dt.float32
F32R = mybir.dt.float32r
BF16 = mybir.dt.bfloat16
FP8 = mybir.dt.float8e5
NPFP8 = ml_dtypes.float8_e5m2
DR = mybir.MatmulPerfMode.DoubleRow
SIGN = mybir.ActivationFunctionType.Sign
GE = mybir.AluOpType.is_ge

N_CORES = 8
B, CIN, DOWN, UP, H, W = 16, 256, 64, 256, 56, 56
HW = H * W               # 3136
PWP = 64                 # padded row pitch (bytes, fp8)
PH = 59                  # 58 real padded rows + 1 slack row
PBUF = PH * PWP          # 3776
IMGS = B // N_CORES      # 2
ROWS = 8
NT = H // ROWS           # 7
NTILE = ROWS * W         # 448
EPS = 1e-4
BIG = 4096.0
WB_BASE = 0              # w8 blob offsets (fp8 cols)
WD_BASE = 1536
WR_BASE = 3072
WC_BASE = 3584
W8_COLS = 3712
H3S = 2.0 ** 16          # h3 residual scale (exact in fp8e5)
CHUNK = 2 * NTILE        # 896-col input DMA chunks == A pair extent

_compiled = None


def _sign(w):
    return np.where(w >= 0, 1.0, -1.0)


def _ov(t_ap, off, dims):
    """Hand-built (possibly overlapping) AP on an SBUF tile."""
    return AP(t_ap.tensor, t_ap.offset + off,
              [list(t_ap.ap[0])] + [list(d) for d in dims])


def _build_nc():
    nc = bacc.Bacc("TRN2", target_bir_lowering=False, debug=False,
                   num_devices=N_CORES)

    # xh: fp16 terms h1,h2 laid [img, part, kh*2+term, col]
    xh = nc.declare_dram_parameter("xh", [IMGS, 128, 4, HW],
                                   mybir.dt.float16, isOutput=False)
    # x3: fp8 h3*2^16 residual laid [img, part, kh, col]
    x3 = nc.declare_dram_parameter("x3", [IMGS, 128, 2, HW], FP8,
                                   isOutput=False)
    wa = nc.declare_dram_parameter("wa", [128, 128], mybir.dt.float16,
                                   isOutput=False)
    w8 = nc.declare_dram_parameter("w8", [128, W8_COLS], FP8, isOutput=False)
    # bn/threshold table f32, columns:
    #  0 inv11 | 1 beta11 (A, 64p) | 2 inv31 lo | 3 beta31 lo (B mh0)
    #  4 tB hi (B mh1 is_ge) | 5 tC (C is_ge, 64p)
    #  6,7 D Sign biases -tau_mh | 8,9 D is_ge thresholds tau_mh
    bnp = nc.declare_dram_parameter("bn", [128, 10], F32, isOutput=False)
    # wa8: phase-A h3 DR weights, separate tiny param so phase A is not
    # gated on the big w8 blob's DMA
    wa8p = nc.declare_dram_parameter("wa8", [128, 128], FP8, isOutput=False)
    y = nc.declare_dram_parameter("y", [IMGS, UP, H // 2, W // 2], BF16,
                                  isOutput=True)

    with tile.TileContext(nc) as tc:
        with (
            tc.tile_pool(name="const", bufs=1) as cpool,
            tc.tile_pool(name="act", bufs=1) as apool,
            tc.tile_pool(name="work", bufs=4) as wpool,
            tc.tile_pool(name="psA", bufs=2, space="PSUM") as psA,
            tc.tile_pool(name="psQ", bufs=6, space="PSUM") as psQ,
        ):
            wa_sb = cpool.tile([128, 128], mybir.dt.float16, tag="wa")
            nc.sync.dma_start(out=wa_sb[:], in_=wa[:])
            bn_sb = cpool.tile([128, 10], F32, tag="bn")
            wa8_sb = cpool.tile([128, 128], FP8, tag="wa8")
            w8_sb = cpool.tile([128, W8_COLS], FP8, tag="w8")

            w1a = [wa_sb[:, kh * 64:kh * 64 + 64] for kh in range(2)]

            def w3(base, mh, j):          # [128, 2, 128] conv3x3 weight block
                o = base + (mh * 3 + j) * 256
                return w8_sb[:, o:o + 256].rearrange("p (two m) -> p two m",
                                                     two=2)

            def wR(mh):                   # [128, 2, 128] resid identity block
                o = WR_BASE + mh * 256
                return w8_sb[:, o:o + 256].rearrange("p (two m) -> p two m",
                                                     two=2)

            wC = w8_sb[:, WC_BASE:WC_BASE + 128].rearrange(
                "p (two m) -> p two m", two=2)
            wA8 = wa8_sb[:].rearrange("p (two m) -> p two m", two=2)

            def bncol(c, p=128):
                return bn_sb[0:p, c:c + 1]

            # ---- persistent activation buffers ----
            xhsb = [apool.tile([128, 4, HW], mybir.dt.float16,
                               tag=f"xh{i}", name=f"xh{i}")
                    for i in range(IMGS)]
            x3sb = [apool.tile([128, 2, HW], FP8, tag=f"x3{i}",
                               name=f"x3{i}") for i in range(IMGS)]
            x1p = [apool.tile([128, PBUF], FP8, tag=f"x1p{i}", name=f"x1p{i}")
                   for i in range(IMGS)]
            x2p = [apool.tile([128, PBUF], FP8, tag=f"x2p{i}", name=f"x2p{i}")
                   for i in range(IMGS)]
            # hbuf plane dim: 0 = ch 0-127 (+-1, ACT), 1 = ch 128-255 (0/1)
            hbuf = [apool.tile([128, 2, HW], FP8, tag=f"h{i}", name=f"h{i}")
                    for i in range(IMGS)]
            # obuf bf16: +-1 on even (t+mh) parity, 0/1 on odd (host maps)
            obuf = [[apool.tile([128, 28 * 28], BF16, tag=f"o{i}{m}",
                                name=f"o{i}{m}") for m in range(2)]
                    for i in range(IMGS)]

            # input DMA: bn+wa first (A deps); img0 fp16 terms per-tile for a
            # fast pipeline head (x3 per-pair: per-tile fp8 chunks are under
            # the 512B full-rate threshold); w8 early (B(0,0) needs it); img1
            # in pair chunks.
            def ldx3(i, h):
                c0, c1 = (0, 1792) if h == 0 else (1792, HW)
                nc.sync.dma_start(out=x3sb[i][:, :, c0:c1],
                                  in_=x3[i][:, :, c0:c1])

            def ldxh(i, c0, c1):
                nc.sync.dma_start(out=xhsb[i][:, :, c0:c1],
                                  in_=xh[i][:, :, c0:c1])

            def ldx0(t):
                ldxh(0, t * NTILE, (t + 1) * NTILE)

            def ldx1(p):
                ldxh(1, p * CHUNK, min((p + 1) * CHUNK, HW))

            # border zeroing: x1p +-1 convention (pad 0), x2p 0/1 (pad 0.5)
            for t, pad in ((x1p[0], 0.0), (x1p[1], 0.0),
                           (x2p[0], 0.5), (x2p[1], 0.5)):
                t3 = t[:].rearrange("p (h w) -> p h w", w=PWP)
                nc.gpsimd.memset(t[:, 0:PWP], pad)                # row 0
                nc.gpsimd.memset(t[:, 57 * PWP:PBUF], pad)        # rows 57-58
                nc.gpsimd.memset(t3[:, 1:57, 0:1], pad)           # col 0
                nc.gpsimd.memset(t3[:, 1:57, 57:59], pad)         # cols 57-58

            # sigma row-chunks [lo, hi): chunk k needs A/C tiles writing rows
            # < hi; B/D tile t reads sigma rows 8t..8t+9. Boundary at 49 (not
            # 50) so chunk 2 depends only on tiles t4/t5, not t6.
            SIGC = ((0, 18), (18, 34), (34, 49), (49, 59))

            def sigma(dst, k):
                # partitions 64-127 <- partitions 0-63 shifted left 1 byte.
                # On the SP queue: DMA_ENGINES is a FIFO, so later input
                # chunks are *called* after the sigmas they must not queue
                # ahead of (see schedule below).
                lo, hi = SIGC[k]
                a = lo * PWP
                b = min(hi * PWP, PBUF - 1)
                nc.sync.dma_start(out=dst[64:128, a:b],
                                  in_=dst[0:64, a + 1:b + 1])

            def conv3(src, base, mh, t, pspool, tag, extra=None):
                """3 DoubleRow matmuls; optional 4th accumulating (resid)."""
                r0 = t * ROWS
                ps = pspool.tile([128, NTILE], F32, tag=tag)
                sap = src[:]
                # mm0: taps (0,0),(0,1),(1,0),(1,1); planes=ky (stride PWP)
                rhs = _ov(sap, r0 * PWP, [[PWP, 2], [PWP, ROWS], [1, W]])
                nc.tensor.matmul(ps[:], w3(base, mh, 0), rhs,
                                 start=True, stop=False, perf_mode=DR)
                # mm1: taps (0,2),(1,2) on parts 0-63; planes=ky
                rhs = _ov(sap, r0 * PWP + 2, [[PWP, 2], [PWP, ROWS], [1, W]])
                nc.tensor.matmul(ps[:], w3(base, mh, 1), rhs,
                                 start=False, stop=False, perf_mode=DR)
                # mm2: taps (2,0),(2,1),(2,2); planes=kx+0/kx+2 (stride 2B)
                rhs = _ov(sap, (r0 + 2) * PWP, [[2, 2], [PWP, ROWS], [1, W]])
                nc.tensor.matmul(ps[:], w3(base, mh, 2), rhs,
                                 start=False, stop=(extra is None),
                                 perf_mode=DR)
                if extra is not None:
                    nc.tensor.matmul(ps[:], wR(mh),
                                     extra[:, 0:2, t * NTILE:(t + 1) * NTILE],
                                     start=False, stop=True, perf_mode=DR)
                return ps

            def phase_A(img, t):
                """Tile t: 4 fp16 matmuls (h1,h2 x kh) + 1 fp8 DR matmul
                (h3 residual, weights +-2^-16), then ACT Sign -> x1p."""
                ps = psA.tile([64, 512], F32, tag="pa")
                c0 = t * NTILE
                for j in range(4):          # j = kh*2 + term
                    nc.tensor.matmul(ps[:, 0:NTILE], w1a[j >> 1],
                                     xhsb[img][:, j, c0:c0 + NTILE],
                                     start=(j == 0), stop=False)
                nc.tensor.matmul(ps[:, 0:NTILE], wA8,
                                 x3sb[img][:, 0:2, c0:c0 + NTILE],
                                 start=False, stop=True, perf_mode=DR)
                d3 = x1p[img][:].rearrange("p (h w) -> p h w", w=PWP)
                r0 = t * ROWS
                nc.scalar.activation(d3[0:64, r0 + 1:r0 + 9, 1:57],
                                     ps[:, 0:NTILE], SIGN, bias=bncol(1, 64),
                                     scale=bncol(0, 64))

            def phase_B(img, t, mh):
                ps = conv3(x1p[img], WB_BASE, mh, t, psQ, "q")
                dst = hbuf[img][:, mh, t * NTILE:(t + 1) * NTILE]
                if mh == 0:
                    nc.scalar.activation(dst, ps[:], SIGN,
                                         bias=bncol(3), scale=bncol(2))
                else:
                    nc.vector.tensor_scalar(out=dst, in0=ps[:],
                                            scalar1=bncol(4), scalar2=None,
                                            op0=GE)

            def phase_C(img, t, eng="pool"):
                """Tile t; is_ge (0/1, pad 0.5) -> x2p on Pool, or DVE for
                the tail tiles (shorter latency off the critical suffix)."""
                ps = psA.tile([64, 512], F32, tag="pa")
                c0 = t * NTILE
                nc.tensor.matmul(ps[:, 0:NTILE], wC,
                                 hbuf[img][:, 0:2, c0:c0 + NTILE],
                                 start=True, stop=True, perf_mode=DR)
                d3 = x2p[img][:].rearrange("p (h w) -> p h w", w=PWP)
                r0 = t * ROWS
                # Pool (GPSIMD) cannot read PSUM (BIR verifier); DVE only.
                e = nc.vector
                e.tensor_scalar(
                    out=d3[0:64, r0 + 1:r0 + 9, 1:57], in0=ps[:, 0:NTILE],
                    scalar1=bncol(5, 64), scalar2=None, op0=GE)

            def phase_D(img, t, mh):
                """conv3x3 + resid into PSUM; binarize (ACT Sign -> +-1 on
                even (t+mh); is_ge -> 0/1 on odd: Pool for early tiles, DVE
                for late ones; TensorTensor cannot read PSUM so threshold
                precedes the pools); 2x2 maxpool on bf16 in SBUF (pool1 gets
                the DVE 2x mode), pool2 writes obuf directly. Host maps
                odd-parity tiles 0/1 -> +-1."""
                ps = conv3(x2p[img], WD_BASE, mh, t, psQ, "q",
                           extra=hbuf[img])
                s = wpool.tile([128, NTILE], BF16, tag="s")
                if (t + mh) % 2 == 0 or t >= 5:
                    # t>=5 also on ACT: keeps the DVE queue clear at the tail
                    nc.scalar.activation(s[:], ps[:], SIGN,
                                         bias=bncol(6 + mh), scale=1.0)
                else:
                    nc.vector.tensor_scalar(out=s[:], in0=ps[:],
                                            scalar1=bncol(8 + mh),
                                            scalar2=None, op0=GE)
                s4 = s[:].rearrange("p (h two w) -> p h two w", two=2, w=W)
                v = wpool.tile([128, 4, W], BF16, tag="v")
                nc.vector.tensor_max(out=v[:], in0=s4[:, :, 0, :],
                                     in1=s4[:, :, 1, :])
                v4 = v[:].rearrange("p h (w two) -> p h w two", two=2)
                ob = obuf[img][mh][:].rearrange("p (h w) -> p h w", w=W // 2)
                nc.vector.tensor_max(out=ob[:, 4 * t:4 * t + 4, :],
                                     in0=v4[:, :, :, 0], in1=v4[:, :, :, 1])

            def store(img, mh, half=None):
                yv = y[img, mh * 128:(mh + 1) * 128].rearrange(
                    "p h w -> p (h w)")
                if half is None:
                    nc.sync.dma_start(out=yv, in_=obuf[img][mh][:])
                else:
                    sl = slice(0, 448) if half == 0 else slice(448, 784)
                    nc.sync.dma_start(out=yv[:, sl],
                                      in_=obuf[img][mh][:, sl])

            # ---------------- schedule ----------------
            # PE program order follows estimated data readiness (img0 chunks
            # ~3-13us, img1 ~14-26us on the serial DMA stream); input-chunk
            # and sigma dma_start calls are interleaved so the DMA_ENGINES
            # FIFO order matches readiness. Sigma chunk k serves B/D tiles
            # {2k-1(upper rows), 2k, 2k+1}.
            def Bt(i, t):
                phase_B(i, t, 0)
                phase_B(i, t, 1)

            def Dt(i, t):
                phase_D(i, t, 0)
                phase_D(i, t, 1)

            ldx0(0)
            ldx3(0, 0)
            nc.sync.dma_start(out=bn_sb[:], in_=bnp[:])
            nc.sync.dma_start(out=wa8_sb[:], in_=wa8p[:])
            ldx0(1)
            ldx0(2)
            ldx0(3)
            nc.sync.dma_start(out=w8_sb[:], in_=w8[:])
            phase_A(0, 0)
            phase_A(0, 1)
            phase_A(0, 2)
            ldx0(4)
            ldx3(0, 1)
            sigma(x1p[0], 0)
            phase_A(0, 3)
            phase_A(0, 4)
            Bt(0, 0)
            Bt(0, 1)
            ldx0(5)
            ldx0(6)
            sigma(x1p[0], 1)
            phase_A(0, 5)
            phase_A(0, 6)
            Bt(0, 2)
            Bt(0, 3)
            ldx1(0)
            ldx3(1, 0)
            sigma(x1p[0], 2)
            sigma(x1p[0], 3)
            Bt(0, 4)
            Bt(0, 5)
            Bt(0, 6)
            for t in range(3):
                phase_C(0, t)
            ldx1(1)
            sigma(x2p[0], 0)
            for t in range(3, 7):
                phase_C(0, t)
            sigma(x2p[0], 1)
            sigma(x2p[0], 2)
            sigma(x2p[0], 3)
            Dt(0, 0)
            Dt(0, 1)
            phase_A(1, 0)
            phase_A(1, 1)
            ldx1(2)
            ldx3(1, 1)
            Dt(0, 2)
            Dt(0, 3)
            Dt(0, 4)
            phase_A(1, 2)
            phase_A(1, 3)
            sigma(x1p[1], 0)
            ldx1(3)
            Dt(0, 5)
            Dt(0, 6)
            phase_A(1, 4)
            phase_A(1, 5)
            sigma(x1p[1], 1)
            Bt(1, 0)
            Bt(1, 1)
            phase_A(1, 6)
            sigma(x1p[1], 2)
            sigma(x1p[1], 3)
            store(0, 0)
            store(0, 1)
            Bt(1, 2)
            Bt(1, 3)
            phase_C(1, 0)
            phase_C(1, 1)
            Bt(1, 4)
            Bt(1, 5)
            sigma(x2p[1], 0)
            phase_C(1, 2)
            phase_C(1, 3)
            Bt(1, 6)
            phase_C(1, 4)
            sigma(x2p[1], 1)
            phase_C(1, 5, "dve")
            Dt(1, 0)
            Dt(1, 1)
            phase_C(1, 6, "dve")
            sigma(x2p[1], 2)
            sigma(x2p[1], 3)
            Dt(1, 2)
            Dt(1, 3)
            Dt(1, 4)
            store(1, 0, 0)
            store(1, 1, 0)
            Dt(1, 5)
            Dt(1, 6)
            store(1, 0, 1)
            store(1, 1, 1)

    nc.compile()
    return nc


def _midpoint(t):
    """Half-integer strictly-equivalent threshold for integer-valued PSUM:
    v >= t  <=>  v >= midpoint(t)."""
    t = np.asarray(t, np.float64)
    up = np.ceil(t)
    up = np.where(up == t, t, up)      # integer t: keep (inclusive >=)
    return up - 0.5


def _host_prep(inputs):
    f64 = np.float64

    def inv_beta(g, b, m, v):
        inv = g.astype(f64) / np.sqrt(v.astype(f64) + EPS)
        return inv, b.astype(f64) - m.astype(f64) * inv

    inv11, beta11 = inv_beta(inputs["g11"], inputs["b11"], inputs["m11"],
                             inputs["v11"])
    inv31, beta31 = inv_beta(inputs["g31"], inputs["b31"], inputs["m31"],
                             inputs["v31"])
    inv12, beta12 = inv_beta(inputs["g12"], inputs["b12"], inputs["m12"],
                             inputs["v12"])
    inv32, beta32 = inv_beta(inputs["g32"], inputs["b32"], inputs["m32"],
                             inputs["v32"])

    W31 = _sign(inputs["w31"])               # [256, 64, 3, 3]
    W12 = _sign(inputs["w12"][:, :, 0, 0])   # [64, 256]
    W32 = _sign(inputs["w32"])

    # ---- w8 blob ----
    w8 = np.zeros((128, W8_COLS), f64)

    def pack3(base, Wf, wscale):
        # 3-matmul conv3x3 packing; lhsT [128, 2 planes, 128 m] per block.
        # mm0: part c pl p -> W[m,c,p,0]; part 64+c pl p -> W[m,c,p,1]
        # mm1: part c pl p -> W[m,c,p,2]; parts 64+ zero
        # mm2: pl0: part c -> W[m,c,2,0], part 64+c -> W[m,c,2,1];
        #      pl1: part c -> W[m,c,2,2], part 64+c -> zero
        for mh in range(2):
            ms = slice(mh * 128, (mh + 1) * 128)
            blk = np.zeros((3, 128, 2, 128), f64)
            for p in range(2):
                blk[0, 0:64, p] = Wf[ms, :, p, 0].T * wscale
                blk[0, 64:128, p] = Wf[ms, :, p, 1].T * wscale
                blk[1, 0:64, p] = Wf[ms, :, p, 2].T * wscale
            blk[2, 0:64, 0] = Wf[ms, :, 2, 0].T * wscale
            blk[2, 64:128, 0] = Wf[ms, :, 2, 1].T * wscale
            blk[2, 0:64, 1] = Wf[ms, :, 2, 2].T * wscale
            for j in range(3):
                o = base + (mh * 3 + j) * 256
                w8[:, o:o + 256] = blk[j].reshape(128, 256)

    pack3(WB_BASE, W31, 1.0)      # B: x1p is +-1
    pack3(WD_BASE, W32, 2.0)      # D: x2p is 0/1 -> weights 2W

    # resid identity: plane mh = scale*I (h mh0 +-1 -> BIG/2; mh1 0/1 -> BIG)
    for mh, s in ((0, BIG / 2), (1, BIG)):
        o = WR_BASE + mh * 256
        blk = np.zeros((128, 2, 128), f64)
        blk[:, mh, :] = np.eye(128) * s
        w8[:, o:o + 256] = blk.reshape(128, 256)

    # C: plane0 ch0-127 (+-1, ACT), plane1 ch128-255 (0/1 DVE, weights 2W)
    wc = np.zeros((128, 2, 64), f64)
    wc[:, 0] = W12[:, 0:128].T
    wc[:, 1] = W12[:, 128:256].T * 2.0
    w8[:, WC_BASE:WC_BASE + 128] = wc.reshape(128, 128)

    # ---- phase A: fp16 weights (+-1) + fp8 DR block (+-2^-16) for h3 ----
    W1 = _sign(inputs["w11"][:, :, 0, 0]).T          # [256, 64]
    wa = np.zeros((128, 128), np.float16)
    for kh in range(2):
        wa[:, kh * 64:kh * 64 + 64] = W1[kh * 128:(kh + 1) * 128]
    wa8 = np.zeros((128, 2, 64), f64)
    for kh in range(2):
        wa8[:, kh, :] = W1[kh * 128:(kh + 1) * 128] / H3S
    wa8 = wa8.reshape(128, 128)
    wa8_8 = wa8.astype(NPFP8)
    assert np.array_equal(wa8_8.astype(f64), wa8), "wa8 not fp8e5-exact"

    # ---- bn/threshold table ----
    bn = np.zeros((128, 10), np.float32)
    bn[0:64, 0] = inv11.astype(np.float32)
    bn[0:64, 1] = beta11.astype(np.float32)
    bn[:, 2] = inv31[0:128].astype(np.float32)
    bn[:, 3] = beta31[0:128].astype(np.float32)
    bn[:, 4] = _midpoint(-beta31[128:256] / inv31[128:256]).astype(np.float32)
    # C (Pool is_ge on raw): raw = W12lo.h+- + 2 W12hi.h01;
    # true = raw - S_hi; r=+1 iff raw >= S_hi - beta12/inv12
    S_hi = W12[:, 128:256].sum(axis=1)
    bn[0:64, 5] = _midpoint(S_hi - beta12 / inv12).astype(np.float32)
    # D (final Sign on pooled q): q = conv(2W,s01) + R; fires iff
    # q >= tau_mh, tau = midpoint(-beta32/inv32) + rowsum(W32) + Rshift
    t32 = _midpoint(-beta32 / inv32)
    rs32 = W32.sum(axis=(1, 2, 3))
    tau = np.stack([t32[0:128] + rs32[0:128] - BIG / 2,
                    t32[128:256] + rs32[128:256]])
    bn[:, 6] = (-tau[0]).astype(np.float32)
    bn[:, 7] = (-tau[1]).astype(np.float32)
    bn[:, 8] = tau[0].astype(np.float32)
    bn[:, 9] = tau[1].astype(np.float32)

    w8_8 = w8.astype(NPFP8)
    assert np.array_equal(w8_8.astype(f64), w8), "w8 not fp8e5-exact"

    # ---- x = h1 + h2 + h3/2^16: fp16 + fp16 + fp8 (exact for this data) ----
    x = inputs["x"].astype(f64).reshape(B, 2, 128, HW)   # [img, kh, part, c]
    h1 = x.astype(np.float16)
    h2 = (x - h1.astype(f64)).astype(np.float16)
    h3 = (x - h1.astype(f64) - h2.astype(f64)) * H3S
    h3_8 = h3.astype(NPFP8)
    recon = h1.astype(f64) + h2.astype(f64) + h3_8.astype(f64) / H3S
    resid = np.abs(x - recon).max()
    assert resid < 4e-8, resid    # same tolerance the v2 3-term split held
    # xh [img, part, kh*2+term, col]; x3 [img, part, kh, col]
    xh = np.empty((B, 128, 4, HW), np.float16)
    x3a = np.empty((B, 128, 2, HW), NPFP8)
    for kh in range(2):
        xh[:, :, kh * 2 + 0] = h1[:, kh]
        xh[:, :, kh * 2 + 1] = h2[:, kh]
        x3a[:, :, kh] = h3_8[:, kh]
    xh = xh.reshape(N_CORES, IMGS, 128, 4, HW)
    x3a = x3a.reshape(N_CORES, IMGS, 128, 2, HW)
    in_maps = []
    for c in range(N_CORES):
        in_maps.append({"xh": xh[c], "x3": x3a[c], "wa": wa, "wa8": wa8_8,
                        "w8": w8_8, "bn": bn})
    return in_maps


def kernel(**inputs):
    global _compiled
    if _compiled is None:
        _compiled = _build_nc()
    in_maps = _host_prep(inputs)
    res = run_bass_kernel_spmd(_compiled, in_maps, list(range(N_CORES))).results
    out = np.concatenate([res[c]["y"] for c in range(N_CORES)], axis=0)
    out = out.astype(np.float32)
    # odd-parity tiles with t<5 hold 0/1 (is_ge engines); map to +-1 (t>=5
    # uses ACT Sign -> already +-1). Tile t covers output rows 4t..4t+4 of
    # channel half mh.
    for mh in range(2):
        for t in range(NT):
            if (t + mh) % 2 == 1 and t < 5:
                sl = out[:, mh * 128:(mh + 1) * 128, 4 * t:4 * t + 4, :]
                np.multiply(sl, 2.0, out=sl)
                np.subtract(sl, 1.0, out=sl)
    return out


# revision 85
# speedup vs baseline: 1.4142x; 1.0230x over previous
"""Trainium2 Bass kernel for nn_ConvBlock_23021024707487 (v3).

Binarized double conv-block + residual + maxpool, data-parallel over batch
across 8 NeuronCores (2 images per core).

v3 strategy (vs v2):
- Phase A (conv1x1 on real x): x = h1 + h2 + h3/2^16 with h1,h2 fp16 and
  h3 fp8e5 (exact for this data, asserted on host): 4 fp16 matmuls + 1
  fp8 DoubleRow matmul per tile (vs 6 fp16), input DMA 8MB vs 9.6MB.
- conv3x3 in 3 DoubleRow matmuls instead of 4: mm0 covers taps
  (ky,kx) in {0,1}x{0,1} (planes=ky via row stride, partitions=kx via
  sigma copy), mm1 covers (0,2),(1,2) on partitions 0-63, mm2 uses
  plane stride of 2 BYTES (kx+0 / kx+2) to cover (2,0),(2,1),(2,2).
- Residual folded into phase-D PSUM via an identity DoubleRow matmul
  (BIG*h); binarize precedes the 2x2 maxpool (bf16 SBUF, DVE 2x mode),
  pool2 writes obuf directly (no separate output pass).
- Engine balance (Pool/GPSIMD cannot read PSUM and its TensorTensor is
  rejected by the BIR verifier, so Pool only does memsets + SWDGE sigma
  DMAs): ACT does A/Bmh0/C-like Sign work + even-parity D binarize; DVE
  does Bmh1/C is_ge (0/1), odd-parity D binarize, and both pools.
- Conventions: x1p +-1 (pad 0), x2p 0/1 (pad 0.5, D weights 2W with
  threshold shift by rowsum(W)), hbuf plane0 +-1 / plane1 0/1 (folded in
  wC and the resid scales). All post-A PSUM values are integers, so
  half-integer thresholds make every comparison exact in fp32.
- DMA: the sim's DMA_ENGINES is a FIFO, so input chunks, sigma copies
  and stores are enqueued in data-readiness order (sigma row-quarters).
- Output y is bf16 (+-1 / 0-1 exact); host converts + maps parity tiles.
"""

import sys

for _p in ("/opt/trn_rl_repo", "/root/.axon_site/_ro/trn_rl_repo"):
    if _p not in sys.path:
        sys.path.insert(0, _p)

import numpy as np
import ml_dtypes

import concourse.bacc as bacc
import concourse.mybir as mybir
from concourse import tile
from concourse.ap import AP
from concourse.bass_utils import run_bass_kernel_spmd

F32 = mybir.dt.float32
F32R = mybir.dt.float32r
BF16 = mybir.dt.bfloat16
FP8 = mybir.dt.float8e5
NPFP8 = ml_dtypes.float8_e5m2
DR = mybir.MatmulPerfMode.DoubleRow
SIGN = mybir.ActivationFunctionType.Sign
GE = mybir.AluOpType.is_ge

N_CORES = 8
B, CIN, DOWN, UP, H, W = 16, 256, 64, 256, 56, 56
HW = H * W               # 3136
PWP = 64                 # padded row pitch (bytes, fp8)
PH = 59                  # 58 real padded rows + 1 slack row
PBUF = PH * PWP          # 3776
IMGS = B // N_CORES      # 2
ROWS = 8
NT = H // ROWS           # 7
NTILE = ROWS * W         # 448
EPS = 1e-4
BIG = 4096.0
WB_BASE = 0              # w8 blob offsets (fp8 cols)
WD_BASE = 1536
WR_BASE = 3072
WC_BASE = 3584
W8_COLS = 3712
H3S = 2.0 ** 16          # h3 residual scale (exact in fp8e5)
CHUNK = 2 * NTILE        # 896-col input DMA chunks == A pair extent

_compiled = None


def _sign(w):
    return np.where(w >= 0, 1.0, -1.0)


def _ov(t_ap, off, dims):
    """Hand-built (possibly overlapping) AP on an SBUF tile."""
    return AP(t_ap.tensor, t_ap.offset + off,
              [list(t_ap.ap[0])] + [list(d) for d in dims])


def _build_nc():
    nc = bacc.Bacc("TRN2", target_bir_lowering=False, debug=False,
                   num_devices=N_CORES)

    # xh: fp16 terms h1,h2 laid [img, part, kh*2+term, col]
    xh = nc.declare_dram_parameter("xh", [IMGS, 128, 4, HW],
                                   mybir.dt.float16, isOutput=False)
    # x3: fp8 h3*2^16 residual laid [img, part, kh, col]
    x3 = nc.declare_dram_parameter("x3", [IMGS, 128, 2, HW], FP8,
                                   isOutput=False)
    wa = nc.declare_dram_parameter("wa", [128, 128], mybir.dt.float16,
                                   isOutput=False)
    w8 = nc.declare_dram_parameter("w8", [128, W8_COLS], FP8, isOutput=False)
    # bn/threshold table f32, columns:
    #  0 inv11 | 1 beta11 (A, 64p) | 2 inv31 lo | 3 beta31 lo (B mh0)
    #  4 tB hi (B mh1 is_ge) | 5 tC (C is_ge, 64p)
    #  6,7 D Sign biases -tau_mh | 8,9 D is_ge thresholds tau_mh
    bnp = nc.declare_dram_parameter("bn", [128, 10], F32, isOutput=False)
    # wa8: phase-A h3 DR weights, separate tiny param so phase A is not
    # gated on the big w8 blob's DMA
    wa8p = nc.declare_dram_parameter("wa8", [128, 128], FP8, isOutput=False)
    y = nc.declare_dram_parameter("y", [IMGS, UP, H // 2, W // 2], BF16,
                                  isOutput=True)

    with tile.TileContext(nc) as tc:
        with (
            tc.tile_pool(name="const", bufs=1) as cpool,
            tc.tile_pool(name="act", bufs=1) as apool,
            tc.tile_pool(name="work", bufs=4) as wpool,
            tc.tile_pool(name="psA", bufs=2, space="PSUM") as psA,
            tc.tile_pool(name="psQ", bufs=6, space="PSUM") as psQ,
        ):
            wa_sb = cpool.tile([128, 128], mybir.dt.float16, tag="wa")
            nc.sync.dma_start(out=wa_sb[:], in_=wa[:])
            bn_sb = cpool.tile([128, 10], F32, tag="bn")
            wa8_sb = cpool.tile([128, 128], FP8, tag="wa8")
            w8_sb = cpool.tile([128, W8_COLS], FP8, tag="w8")

            w1a = [wa_sb[:, kh * 64:kh * 64 + 64] for kh in range(2)]

            def w3(base, mh, j):          # [128, 2, 128] conv3x3 weight block
                o = base + (mh * 3 + j) * 256
                return w8_sb[:, o:o + 256].rearrange("p (two m) -> p two m",
                                                     two=2)

            def wR(mh):                   # [128, 2, 128] resid identity block
                o = WR_BASE + mh * 256
                return w8_sb[:, o:o + 256].rearrange("p (two m) -> p two m",
                                                     two=2)

            wC = w8_sb[:, WC_BASE:WC_BASE + 128].rearrange(
                "p (two m) -> p two m", two=2)
            wA8 = wa8_sb[:].rearrange("p (two m) -> p two m", two=2)

            def bncol(c, p=128):
                return bn_sb[0:p, c:c + 1]

            # ---- persistent activation buffers ----
            xhsb = [apool.tile([128, 4, HW], mybir.dt.float16,
                               tag=f"xh{i}", name=f"xh{i}")
                    for i in range(IMGS)]
            x3sb = [apool.tile([128, 2, HW], FP8, tag=f"x3{i}",
                               name=f"x3{i}") for i in range(IMGS)]
            x1p = [apool.tile([128, PBUF], FP8, tag=f"x1p{i}", name=f"x1p{i}")
                   for i in range(IMGS)]
            x2p = [apool.tile([128, PBUF], FP8, tag=f"x2p{i}", name=f"x2p{i}")
                   for i in range(IMGS)]
            # hbuf plane dim: 0 = ch 0-127 (+-1, ACT), 1 = ch 128-255 (0/1)
            hbuf = [apool.tile([128, 2, HW], FP8, tag=f"h{i}", name=f"h{i}")
                    for i in range(IMGS)]
            # obuf bf16: +-1 on even (t+mh) parity, 0/1 on odd (host maps)
            obuf = [[apool.tile([128, 28 * 28], BF16, tag=f"o{i}{m}",
                                name=f"o{i}{m}") for m in range(2)]
                    for i in range(IMGS)]

            # input DMA: bn+wa first (A deps); img0 fp16 terms per-tile for a
            # fast pipeline head (x3 per-pair: per-tile fp8 chunks are under
            # the 512B full-rate threshold); w8 early (B(0,0) needs it); img1
            # in pair chunks.
            def ldx3(i, h):
                c0, c1 = (0, 1792) if h == 0 else (1792, HW)
                nc.sync.dma_start(out=x3sb[i][:, :, c0:c1],
                                  in_=x3[i][:, :, c0:c1])

            def ldxh(i, c0, c1):
                nc.sync.dma_start(out=xhsb[i][:, :, c0:c1],
                                  in_=xh[i][:, :, c0:c1])

            def ldx0(t):
                ldxh(0, t * NTILE, (t + 1) * NTILE)

            def ldx1(p):
                ldxh(1, p * CHUNK, min((p + 1) * CHUNK, HW))

            # border zeroing: x1p +-1 convention (pad 0), x2p 0/1 (pad 0.5)
            for t, pad in ((x1p[0], 0.0), (x1p[1], 0.0),
                           (x2p[0], 0.5), (x2p[1], 0.5)):
                t3 = t[:].rearrange("p (h w) -> p h w", w=PWP)
                nc.gpsimd.memset(t[:, 0:PWP], pad)                # row 0
                nc.gpsimd.memset(t[:, 57 * PWP:PBUF], pad)        # rows 57-58
                nc.gpsimd.memset(t3[:, 1:57, 0:1], pad)           # col 0
                nc.gpsimd.memset(t3[:, 1:57, 57:59], pad)         # cols 57-58

            # sigma row-chunks [lo, hi): chunk k needs A/C tiles writing rows
            # < hi; B/D tile t reads sigma rows 8t..8t+9. Boundary at 49 (not
            # 50) so chunk 2 depends only on tiles t4/t5, not t6.
            SIGC = ((0, 18), (18, 34), (34, 49), (49, 59))

            def sigma(dst, k):
                # partitions 64-127 <- partitions 0-63 shifted left 1 byte.
                # On the SP queue: DMA_ENGINES is a FIFO, so later input
                # chunks are *called* after the sigmas they must not queue
                # ahead of (see schedule below).
                lo, hi = SIGC[k]
                a = lo * PWP
                b = min(hi * PWP, PBUF - 1)
                nc.sync.dma_start(out=dst[64:128, a:b],
                                  in_=dst[0:64, a + 1:b + 1])

            def conv3(src, base, mh, t, pspool, tag, extra=None):
                """3 DoubleRow matmuls; optional 4th accumulating (resid)."""
                r0 = t * ROWS
                ps = pspool.tile([128, NTILE], F32, tag=tag)
                sap = src[:]
                # mm0: taps (0,0),(0,1),(1,0),(1,1); planes=ky (stride PWP)
                rhs = _ov(sap, r0 * PWP, [[PWP, 2], [PWP, ROWS], [1, W]])
                nc.tensor.matmul(ps[:], w3(base, mh, 0), rhs,
                                 start=True, stop=False, perf_mode=DR)
                # mm1: taps (0,2),(1,2) on parts 0-63; planes=ky
                rhs = _ov(sap, r0 * PWP + 2, [[PWP, 2], [PWP, ROWS], [1, W]])
                nc.tensor.matmul(ps[:], w3(base, mh, 1), rhs,
                                 start=False, stop=False, perf_mode=DR)
                # mm2: taps (2,0),(2,1),(2,2); planes=kx+0/kx+2 (stride 2B)
                rhs = _ov(sap, (r0 + 2) * PWP, [[2, 2], [PWP, ROWS], [1, W]])
                nc.tensor.matmul(ps[:], w3(base, mh, 2), rhs,
                                 start=False, stop=(extra is None),
                                 perf_mode=DR)
                if extra is not None:
                    nc.tensor.matmul(ps[:], wR(mh),
                                     extra[:, 0:2, t * NTILE:(t + 1) * NTILE],
                                     start=False, stop=True, perf_mode=DR)
                return ps

            def phase_A(img, t):
                """Tile t: 4 fp16 matmuls (h1,h2 x kh) + 1 fp8 DR matmul
                (h3 residual, weights +-2^-16), then ACT Sign -> x1p."""
                ps = psA.tile([64, 512], F32, tag="pa")
                c0 = t * NTILE
                for j in range(4):          # j = kh*2 + term
                    nc.tensor.matmul(ps[:, 0:NTILE], w1a[j >> 1],
                                     xhsb[img][:, j, c0:c0 + NTILE],
                                     start=(j == 0), stop=False)
                nc.tensor.matmul(ps[:, 0:NTILE], wA8,
                                 x3sb[img][:, 0:2, c0:c0 + NTILE],
                                 start=False, stop=True, perf_mode=DR)
                d3 = x1p[img][:].rearrange("p (h w) -> p h w", w=PWP)
                r0 = t * ROWS
                nc.scalar.activation(d3[0:64, r0 + 1:r0 + 9, 1:57],
                                     ps[:, 0:NTILE], SIGN, bias=bncol(1, 64),
                                     scale=bncol(0, 64))

            def phase_B(img, t, mh):
                ps = conv3(x1p[img], WB_BASE, mh, t, psQ, "q")
                dst = hbuf[img][:, mh, t * NTILE:(t + 1) * NTILE]
                if mh == 0:
                    nc.scalar.activation(dst, ps[:], SIGN,
                                         bias=bncol(3), scale=bncol(2))
                else:
                    nc.vector.tensor_scalar(out=dst, in0=ps[:],
                                            scalar1=bncol(4), scalar2=None,
                                            op0=GE)

            def phase_C(img, t, eng="pool"):
                """Tile t; is_ge (0/1, pad 0.5) -> x2p on Pool, or DVE for
                the tail tiles (shorter latency off the critical suffix)."""
                ps = psA.tile([64, 512], F32, tag="pa")
                c0 = t * NTILE
                nc.tensor.matmul(ps[:, 0:NTILE], wC,
                                 hbuf[img][:, 0:2, c0:c0 + NTILE],
                                 start=True, stop=True, perf_mode=DR)
                d3 = x2p[img][:].rearrange("p (h w) -> p h w", w=PWP)
                r0 = t * ROWS
                # Pool (GPSIMD) cannot read PSUM (BIR verifier); DVE only.
                e = nc.vector
                e.tensor_scalar(
                    out=d3[0:64, r0 + 1:r0 + 9, 1:57], in0=ps[:, 0:NTILE],
                    scalar1=bncol(5, 64), scalar2=None, op0=GE)

            def phase_D(img, t, mh):
                """conv3x3 + resid into PSUM; binarize (ACT Sign -> +-1 on
                even (t+mh); is_ge -> 0/1 on odd: Pool for early tiles, DVE
                for late ones; TensorTensor cannot read PSUM so threshold
                precedes the pools); 2x2 maxpool on bf16 in SBUF (pool1 gets
                the DVE 2x mode), pool2 writes obuf directly. Host maps
                odd-parity tiles 0/1 -> +-1."""
                ps = conv3(x2p[img], WD_BASE, mh, t, psQ, "q",
                           extra=hbuf[img])
                s = wpool.tile([128, NTILE], BF16, tag="s")
                if (t + mh) % 2 == 0 or t >= 5:
                    # t>=5 also on ACT: keeps the DVE queue clear at the tail
                    nc.scalar.activation(s[:], ps[:], SIGN,
                                         bias=bncol(6 + mh), scale=1.0)
                else:
                    nc.vector.tensor_scalar(out=s[:], in0=ps[:],
                                            scalar1=bncol(8 + mh),
                                            scalar2=None, op0=GE)
                s4 = s[:].rearrange("p (h two w) -> p h two w", two=2, w=W)
                v = wpool.tile([128, 4, W], BF16, tag="v")
                nc.vector.tensor_max(out=v[:], in0=s4[:, :, 0, :],
                                     in1=s4[:, :, 1, :])
                v4 = v[:].rearrange("p h (w two) -> p h w two", two=2)
                ob = obuf[img][mh][:].rearrange("p (h w) -> p h w", w=W // 2)
                nc.vector.tensor_max(out=ob[:, 4 * t:4 * t + 4, :],
                                     in0=v4[:, :, :, 0], in1=v4[:, :, :, 1])

            def store(img, mh, half=None):
                yv = y[img, mh * 128:(mh + 1) * 128].rearrange(
                    "p h w -> p (h w)")
                if half is None:
                    nc.sync.dma_start(out=yv, in_=obuf[img][mh][:])
                else:
                    sl = slice(0, 448) if half == 0 else slice(448, 784)
                    nc.sync.dma_start(out=yv[:, sl],
                                      in_=obuf[img][mh][:, sl])

            # ---------------- schedule ----------------
            # PE program order follows estimated data readiness (img0 chunks
            # ~3-13us, img1 ~14-26us on the serial DMA stream); input-chunk
            # and sigma dma_start calls are interleaved so the DMA_ENGINES
            # FIFO order matches readiness. Sigma chunk k serves B/D tiles
            # {2k-1(upper rows), 2k, 2k+1}.
            def Bt(i, t):
                phase_B(i, t, 0)
                phase_B(i, t, 1)

            def Dt(i, t):
                phase_D(i, t, 0)
                phase_D(i, t, 1)

            ldx0(0)
            ldx3(0, 0)
            nc.sync.dma_start(out=bn_sb[:], in_=bnp[:])
            nc.sync.dma_start(out=wa8_sb[:], in_=wa8p[:])
            ldx0(1)
            ldx0(2)
            ldx0(3)
            nc.sync.dma_start(out=w8_sb[:], in_=w8[:])
            phase_A(0, 0)
            phase_A(0, 1)
            phase_A(0, 2)
            ldx0(4)
            ldx3(0, 1)
            sigma(x1p[0], 0)
            phase_A(0, 3)
            phase_A(0, 4)
            Bt(0, 0)
            Bt(0, 1)
            ldx0(5)
            ldx0(6)
            sigma(x1p[0], 1)
            phase_A(0, 5)
            phase_A(0, 6)
            Bt(0, 2)
            Bt(0, 3)
            sigma(x1p[0], 2)
            ldx1(0)
            sigma(x1p[0], 3)
            ldx3(1, 0)
            Bt(0, 4)
            Bt(0, 5)
            Bt(0, 6)
            for t in range(3):
                phase_C(0, t)
            sigma(x2p[0], 0)
            ldx1(1)
            for t in range(3, 7):
                phase_C(0, t)
            sigma(x2p[0], 1)
            sigma(x2p[0], 2)
            sigma(x2p[0], 3)
            Dt(0, 0)
            Dt(0, 1)
            phase_A(1, 0)
            phase_A(1, 1)
            ldx1(2)
            ldx3(1, 1)
            Dt(0, 2)
            Dt(0, 3)
            Dt(0, 4)
            phase_A(1, 2)
            phase_A(1, 3)
            sigma(x1p[1], 0)
            ldx1(3)
            Dt(0, 5)
            Dt(0, 6)
            phase_A(1, 4)
            phase_A(1, 5)
            sigma(x1p[1], 1)
            sigma(x1p[1], 2)
            Bt(1, 0)
            Bt(1, 1)
            phase_A(1, 6)
            sigma(x1p[1], 3)
            store(0, 0)
            store(0, 1)
            Bt(1, 2)
            Bt(1, 3)
            phase_C(1, 0)
            phase_C(1, 1)
            Bt(1, 4)
            Bt(1, 5)
            sigma(x2p[1], 0)
            phase_C(1, 2)
            phase_C(1, 3)
            Bt(1, 6)
            phase_C(1, 4)
            sigma(x2p[1], 1)
            phase_C(1, 5, "dve")
            sigma(x2p[1], 2)
            Dt(1, 0)
            Dt(1, 1)
            phase_C(1, 6, "dve")
            sigma(x2p[1], 3)
            Dt(1, 2)
            Dt(1, 3)
            Dt(1, 4)
            store(1, 0, 0)
            store(1, 1, 0)
            Dt(1, 5)
            Dt(1, 6)
            store(1, 0, 1)
            store(1, 1, 1)

    nc.compile()
    return nc


def _midpoint(t):
    """Half-integer strictly-equivalent threshold for integer-valued PSUM:
    v >= t  <=>  v >= midpoint(t)."""
    t = np.asarray(t, np.float64)
    up = np.ceil(t)
    up = np.where(up == t, t, up)      # integer t: keep (inclusive >=)
    return up - 0.5


def _host_prep(inputs):
    f64 = np.float64

    def inv_beta(g, b, m, v):
        inv = g.astype(f64) / np.sqrt(v.astype(f64) + EPS)
        return inv, b.astype(f64) - m.astype(f64) * inv

    inv11, beta11 = inv_beta(inputs["g11"], inputs["b11"], inputs["m11"],
                             inputs["v11"])
    inv31, beta31 = inv_beta(inputs["g31"], inputs["b31"], inputs["m31"],
                             inputs["v31"])
    inv12, beta12 = inv_beta(inputs["g12"], inputs["b12"], inputs["m12"],
                             inputs["v12"])
    inv32, beta32 = inv_beta(inputs["g32"], inputs["b32"], inputs["m32"],
                             inputs["v32"])

    W31 = _sign(inputs["w31"])               # [256, 64, 3, 3]
    W12 = _sign(inputs["w12"][:, :, 0, 0])   # [64, 256]
    W32 = _sign(inputs["w32"])

    # ---- w8 blob ----
    w8 = np.zeros((128, W8_COLS), f64)

    def pack3(base, Wf, wscale):
        # 3-matmul conv3x3 packing; lhsT [128, 2 planes, 128 m] per block.
        # mm0: part c pl p -> W[m,c,p,0]; part 64+c pl p -> W[m,c,p,1]
        # mm1: part c pl p -> W[m,c,p,2]; parts 64+ zero
        # mm2: pl0: part c -> W[m,c,2,0], part 64+c -> W[m,c,2,1];
        #      pl1: part c -> W[m,c,2,2], part 64+c -> zero
        for mh in range(2):
            ms = slice(mh * 128, (mh + 1) * 128)
            blk = np.zeros((3, 128, 2, 128), f64)
            for p in range(2):
                blk[0, 0:64, p] = Wf[ms, :, p, 0].T * wscale
                blk[0, 64:128, p] = Wf[ms, :, p, 1].T * wscale
                blk[1, 0:64, p] = Wf[ms, :, p, 2].T * wscale
            blk[2, 0:64, 0] = Wf[ms, :, 2, 0].T * wscale
            blk[2, 64:128, 0] = Wf[ms, :, 2, 1].T * wscale
            blk[2, 0:64, 1] = Wf[ms, :, 2, 2].T * wscale
            for j in range(3):
                o = base + (mh * 3 + j) * 256
                w8[:, o:o + 256] = blk[j].reshape(128, 256)

    pack3(WB_BASE, W31, 1.0)      # B: x1p is +-1
    pack3(WD_BASE, W32, 2.0)      # D: x2p is 0/1 -> weights 2W

    # resid identity: plane mh = scale*I (h mh0 +-1 -> BIG/2; mh1 0/1 -> BIG)
    for mh, s in ((0, BIG / 2), (1, BIG)):
        o = WR_BASE + mh * 256
        blk = np.zeros((128, 2, 128), f64)
        blk[:, mh, :] = np.eye(128) * s
        w8[:, o:o + 256] = blk.reshape(128, 256)

    # C: plane0 ch0-127 (+-1, ACT), plane1 ch128-255 (0/1 DVE, weights 2W)
    wc = np.zeros((128, 2, 64), f64)
    wc[:, 0] = W12[:, 0:128].T
    wc[:, 1] = W12[:, 128:256].T * 2.0
    w8[:, WC_BASE:WC_BASE + 128] = wc.reshape(128, 128)

    # ---- phase A: fp16 weights (+-1) + fp8 DR block (+-2^-16) for h3 ----
    W1 = _sign(inputs["w11"][:, :, 0, 0]).T          # [256, 64]
    wa = np.zeros((128, 128), np.float16)
    for kh in range(2):
        wa[:, kh * 64:kh * 64 + 64] = W1[kh * 128:(kh + 1) * 128]
    wa8 = np.zeros((128, 2, 64), f64)
    for kh in range(2):
        wa8[:, kh, :] = W1[kh * 128:(kh + 1) * 128] / H3S
    wa8 = wa8.reshape(128, 128)
    wa8_8 = wa8.astype(NPFP8)
    assert np.array_equal(wa8_8.astype(f64), wa8), "wa8 not fp8e5-exact"

    # ---- bn/threshold table ----
    bn = np.zeros((128, 10), np.float32)
    bn[0:64, 0] = inv11.astype(np.float32)
    bn[0:64, 1] = beta11.astype(np.float32)
    bn[:, 2] = inv31[0:128].astype(np.float32)
    bn[:, 3] = beta31[0:128].astype(np.float32)
    bn[:, 4] = _midpoint(-beta31[128:256] / inv31[128:256]).astype(np.float32)
    # C (Pool is_ge on raw): raw = W12lo.h+- + 2 W12hi.h01;
    # true = raw - S_hi; r=+1 iff raw >= S_hi - beta12/inv12
    S_hi = W12[:, 128:256].sum(axis=1)
    bn[0:64, 5] = _midpoint(S_hi - beta12 / inv12).astype(np.float32)
    # D (final Sign on pooled q): q = conv(2W,s01) + R; fires iff
    # q >= tau_mh, tau = midpoint(-beta32/inv32) + rowsum(W32) + Rshift
    t32 = _midpoint(-beta32 / inv32)
    rs32 = W32.sum(axis=(1, 2, 3))
    tau = np.stack([t32[0:128] + rs32[0:128] - BIG / 2,
                    t32[128:256] + rs32[128:256]])
    bn[:, 6] = (-tau[0]).astype(np.float32)
    bn[:, 7] = (-tau[1]).astype(np.float32)
    bn[:, 8] = tau[0].astype(np.float32)
    bn[:, 9] = tau[1].astype(np.float32)

    w8_8 = w8.astype(NPFP8)
    assert np.array_equal(w8_8.astype(f64), w8), "w8 not fp8e5-exact"

    # ---- x = h1 + h2 + h3/2^16: fp16 + fp16 + fp8 (exact for this data) ----
    x = inputs["x"].astype(f64).reshape(B, 2, 128, HW)   # [img, kh, part, c]
    h1 = x.astype(np.float16)
    h2 = (x - h1.astype(f64)).astype(np.float16)
    h3 = (x - h1.astype(f64) - h2.astype(f64)) * H3S
    h3_8 = h3.astype(NPFP8)
    recon = h1.astype(f64) + h2.astype(f64) + h3_8.astype(f64) / H3S
    resid = np.abs(x - recon).max()
    assert resid < 4e-8, resid    # same tolerance the v2 3-term split held
    # xh [img, part, kh*2+term, col]; x3 [img, part, kh, col]
    xh = np.empty((B, 128, 4, HW), np.float16)
    x3a = np.empty((B, 128, 2, HW), NPFP8)
    for kh in range(2):
        xh[:, :, kh * 2 + 0] = h1[:, kh]
        xh[:, :, kh * 2 + 1] = h2[:, kh]
        x3a[:, :, kh] = h3_8[:, kh]
    xh = xh.reshape(N_CORES, IMGS, 128, 4, HW)
    x3a = x3a.reshape(N_CORES, IMGS, 128, 2, HW)
    in_maps = []
    for c in range(N_CORES):
        in_maps.append({"xh": xh[c], "x3": x3a[c], "wa": wa, "wa8": wa8_8,
                        "w8": w8_8, "bn": bn})
    return in_maps


def kernel(**inputs):
    global _compiled
    if _compiled is None:
        _compiled = _build_nc()
    in_maps = _host_prep(inputs)
    res = run_bass_kernel_spmd(_compiled, in_maps, list(range(N_CORES))).results
    out = np.concatenate([res[c]["y"] for c in range(N_CORES)], axis=0)
    out = out.astype(np.float32)
    # odd-parity tiles with t<5 hold 0/1 (is_ge engines); map to +-1 (t>=5
    # uses ACT Sign -> already +-1). Tile t covers output rows 4t..4t+4 of
    # channel half mh.
    for mh in range(2):
        for t in range(NT):
            if (t + mh) % 2 == 1 and t < 5:
                sl = out[:, mh * 128:(mh + 1) * 128, 4 * t:4 * t + 4, :]
                np.multiply(sl, 2.0, out=sl)
                np.subtract(sl, 1.0, out=sl)
    return out
